# revision 1
# baseline (speedup 1.0000x reference)
"""Trainium2 Bass kernel for AlphaFold-style gated attention.

Reference math (B=4, N=1024, C=512, H=8, CH=64):
    q = (q_x @ Wq) / 8 ; k = kv_x @ Wk ; v = kv_x @ Wv
    s = q k^T + bias_mask[b,k] + bias_pair[h,q,k]
    a = softmax_k(s) ; o = a @ v
    g = sigmoid(q_x @ Wg + bg)
    out = (o*g) @ Wo + bo

Sharding: 8 cores = (batch b in 0..3) x (q-half qh in 0..1). Zero collectives.

Device-side trick sheet:
  - All activations kept transposed ([feat on partitions, rows on free]);
    host pre-transposes inputs, so no on-chip transposes at all.
  - exp without max subtraction (scores are O(5), fp32 exp is safe).
  - bias_mask folded into v on host: kvem = kv_x * exp(mask)[k].
  - bias_pair folded as host-precomputed exp(pair)^T, multiplied into exp(s).
  - softmax denominator = extra em column in v -> free row in AV matmul.
  - 1/d broadcast across partitions via K=1 outer-product matmul into PSUM.
  - head PAIRS processed together: the two 64-contraction score matmuls use
    PE row groups 0-1/2-3 (concurrent on HW), share one [128,1024] PSUM
    tile, one Exp and one pair-multiply.
  - touch ops keep every instruction at <=1 foreign-semaphore wait.
"""

import sys

import numpy as np

if "/opt/trn_rl_repo" not in sys.path:
    sys.path.insert(0, "/opt/trn_rl_repo")

import ml_dtypes

import concourse.bass as bass
import concourse.tile as tile
from concourse import bacc, mybir
from concourse.bass_utils import run_bass_kernel_spmd

B, N, C, H, CH = 4, 1024, 512, 8, 64
R = 512          # q rows per core
KC = N // 128    # 8 k chunks of 128
CC = C // 128    # 4 feature chunks of 128
F32 = mybir.dt.float32
BF16 = mybir.dt.bfloat16
BF = ml_dtypes.bfloat16


def build(finalize=True):
    nc = bacc.Bacc("TRN2", target_bir_lowering=False, debug=False)

    qxt = nc.dram_tensor("qxt", [C, R], BF16, kind="ExternalInput").ap()
    kvt = nc.dram_tensor("kvt", [C, N], BF16, kind="ExternalInput").ap()
    kvem = nc.dram_tensor("kvem", [C, N], BF16, kind="ExternalInput").ap()
    emb = nc.dram_tensor("emb", [128, KC], BF16, kind="ExternalInput").ap()
    pairt = nc.dram_tensor("pairt", [H, N, R], BF16, kind="ExternalInput").ap()
    wq = nc.dram_tensor("wq", [C, C], BF16, kind="ExternalInput").ap()
    wk = nc.dram_tensor("wk", [C, C], BF16, kind="ExternalInput").ap()
    wv = nc.dram_tensor("wv", [C, C], BF16, kind="ExternalInput").ap()
    wg = nc.dram_tensor("wg", [C, C], BF16, kind="ExternalInput").ap()
    wo = nc.dram_tensor("wo", [C, C], BF16, kind="ExternalInput").ap()
    bgr = nc.dram_tensor("bgr", [128, CC], F32, kind="ExternalInput").ap()
    bor = nc.dram_tensor("bor", [128, CC], F32, kind="ExternalInput").ap()
    out = nc.dram_tensor("out", [C, R], BF16, kind="ExternalOutput").ap()

    with tile.TileContext(nc) as tc:
        _body(tc, qxt, kvt, kvem, emb, pairt, wq, wk, wv, wg, wo, bgr, bor, out)
    if finalize:
        nc.finalize()
    return nc


def _body(tc, qxt, kvt, kvem, emb, pairt, wq, wk, wv, wg, wo, bgr, bor, out):
    nc = tc.nc
    Exp = mybir.ActivationFunctionType.Exp
    Sigmoid = mybir.ActivationFunctionType.Sigmoid
    Ident = mybir.ActivationFunctionType.Identity

    with (
        tc.tile_pool(name="keep", bufs=1) as keep,
        tc.tile_pool(name="sb", bufs=3) as sb,
        tc.tile_pool(name="pairp", bufs=2) as pairp,
        tc.tile_pool(name="dp", bufs=3) as dp,
        tc.tile_pool(name="outp", bufs=2) as outp,
        tc.tile_pool(name="psA", bufs=2, space="PSUM") as psA,
        tc.tile_pool(name="psO", bufs=2, space="PSUM") as psO,
        tc.tile_pool(name="psD", bufs=1, space="PSUM") as psD,
    ):
        # ---- issue input DMAs (q-projection inputs first) ----
        w_sb = {}
        for wname in ("wq", "wk", "wv", "wg", "wo"):
            w_sb[wname] = keep.tile([128, CC, C], BF16, tag=wname,
                                    name=f"w_{wname}")
        qxt_sb = keep.tile([128, CC, R], BF16, tag="qxt")
        kvt_sb = keep.tile([128, CC, N], BF16, tag="kvt")
        kvem_sb = keep.tile([128, CC, N], BF16, tag="kvem")
        bgr_sb = keep.tile([128, CC], F32, tag="bgr")
        bor_sb = keep.tile([128, CC], F32, tag="bor")

        qxt_r = qxt.rearrange("(cc p) r -> cc p r", p=128)
        wq_r = wq.rearrange("(cc p) o -> cc p o", p=128)
        for ci in range(CC):
            nc.sync.dma_start(out=qxt_sb[:, ci, :], in_=qxt_r[ci])
            nc.sync.dma_start(out=w_sb["wq"][:, ci, :], in_=wq_r[ci])
        nc.sync.dma_start(out=kvt_sb, in_=kvt.rearrange("(cc p) n -> p cc n", p=128))
        nc.sync.dma_start(out=w_sb["wk"], in_=wk.rearrange("(cc p) o -> p cc o", p=128))
        nc.sync.dma_start(out=w_sb["wg"], in_=wg.rearrange("(cc p) o -> p cc o", p=128))
        nc.sync.dma_start(out=kvem_sb, in_=kvem.rearrange("(cc p) n -> p cc n", p=128))
        nc.sync.dma_start(out=w_sb["wv"], in_=wv.rearrange("(cc p) o -> p cc o", p=128))
        nc.sync.dma_start(out=bgr_sb, in_=bgr)
        nc.sync.dma_start(out=bor_sb, in_=bor)

        ones_64 = keep.tile([1, 64], BF16, tag="ones_64")
        nc.vector.memset(ones_64, 1.0)
        tps = psD.tile([1, 8], F32, tag="touch")
        scr = keep.tile([1, 2], F32, tag="scr")
        scr2 = keep.tile([1, 1], BF16, tag="scr2")

        qT = keep.tile([128, CC, R], BF16, tag="qT")
        kT = keep.tile([128, CC, N], BF16, tag="kT")
        gT = keep.tile([128, CC, R], F32, tag="gT")
        vS = keep.tile([128, KC, H, 65], BF16, tag="vS")

        # ---- q projection (per-chunk DMAs: each matmul waits <=2 lanes) ----
        for cc in range(CC):
            ps = psA.tile([128, R], F32, tag="ps")
            for ci in range(CC):
                nc.tensor.matmul(
                    ps, w_sb["wq"][:, ci, cc * 128:(cc + 1) * 128],
                    qxt_sb[:, ci, :], start=(ci == 0), stop=(ci == CC - 1))
            nc.vector.tensor_copy(qT[:, cc, :], ps)

        # ---- k projection ----
        nc.tensor.matmul(tps[0:1, 1:2], w_sb["wk"][0:1, 0, 0:1],
                         kvt_sb[0:1, 0, 0:1], start=True, stop=True)
        for cc in range(CC):
            for nh in range(2):
                ps2 = psA.tile([128, R], F32, tag="ps")
                for ci in range(CC):
                    nc.tensor.matmul(
                        ps2, w_sb["wk"][:, ci, cc * 128:(cc + 1) * 128],
                        kvt_sb[:, ci, nh * 512:(nh + 1) * 512],
                        start=(ci == 0), stop=(ci == CC - 1))
                nc.vector.tensor_copy(kT[:, cc, nh * 512:(nh + 1) * 512], ps2)

        # ---- g projection (sigmoid with bias) ----
        nc.tensor.matmul(tps[0:1, 2:3], w_sb["wg"][0:1, 0, 0:1],
                         qxt_sb[0:1, 0, 0:1], start=True, stop=True)
        nc.scalar.activation(scr[0:1, 0:1], bgr_sb[0:1, 0:1], Ident)
        nc.scalar.activation(scr[0:1, 1:2], bor_sb[0:1, 0:1], Ident)
        for cc in range(CC):
            ps3 = psA.tile([128, R], F32, tag="ps")
            for ci in range(CC):
                nc.tensor.matmul(
                    ps3, w_sb["wg"][:, ci, cc * 128:(cc + 1) * 128],
                    qxt_sb[:, ci, :], start=(ci == 0), stop=(ci == CC - 1))
            nc.scalar.activation(gT[:, cc, :], ps3, Sigmoid,
                                 bias=bgr_sb[:, cc:cc + 1])

        # ---- pairt double-buffer prefetch (pair 0 queued before v-proj) ----
        pair_tiles = {}

        def fetch_pair(hp):
            h0 = 2 * hp
            t = pairp.tile([128, KC, 2, R], BF16, tag="pair",
                           name=f"pairt_{hp}")
            for hi in range(2):
                nc.sync.dma_start(
                    out=t[:, :, hi, :],
                    in_=pairt[h0 + hi].rearrange("(kc p) r -> p kc r", p=128))
            pair_tiles[hp] = t

        fetch_pair(0)
        nc.sync.dma_start(out=w_sb["wo"], in_=wo.rearrange("(cc p) o -> p cc o", p=128))

        # ---- v (natural layout from em-scaled kv_x), em column via DMA ----
        nc.tensor.matmul(tps[0:1, 3:4], w_sb["wv"][0:1, 0, 0:1],
                         kvem_sb[0:1, 0, 0:1], start=True, stop=True)
        for kc in range(KC):
            ps = psA.tile([128, R], F32, tag="ps")
            for ci in range(CC):
                nc.tensor.matmul(
                    ps, kvem_sb[:, ci, kc * 128:(kc + 1) * 128],
                    w_sb["wv"][:, ci, :], start=(ci == 0), stop=(ci == CC - 1))
            nc.vector.tensor_copy(vS[:, kc, :, 0:64], ps)
        emb_bcast = bass.AP(tensor=emb.tensor, offset=emb.offset,
                            ap=[[KC, 128], [1, KC], [0, H]])
        nc.gpsimd.dma_start(out=vS[:, :, :, 64:65], in_=emb_bcast)

        # ---- attention, head pairs (row groups 0-1 / 2-3 on PE) ----
        # Software-pipelined: scores for kc issue one step ahead of the AV
        # matmuls (PE is in-order; the lookahead hides the exp+mult chain),
        # and each pair's normalize/gate epilogue is deferred past the next
        # pair's first scores so the db matmuls never stall the PE.
        xgT = keep.tile([128, CC, R], BF16, tag="xgT")
        pending = None

        def emit_epilogue(hp, ov0, ov1, dinvs):
            for hi, ov, dinv in ((0, ov0, dinvs[0]), (1, ov1, dinvs[1])):
                po = hi * 64
                db = psD.tile([64, R], F32, tag="db", name=f"db_{hp}_{hi}")
                nc.tensor.matmul(db, ones_64, dinv, start=True, stop=True)
                gd = dp.tile([64, R], F32, tag="gd", name=f"gd_{hp}_{hi}")
                nc.vector.tensor_mul(gd, gT[po:po + 64, hp, :], db)
                nc.vector.tensor_mul(xgT[po:po + 64, hp, :], ov[0:64, :], gd)

        for hp in range(H // 2):
            h0 = 2 * hp
            pairt_h = pair_tiles[hp]
            nc.vector.tensor_copy(scr2, pairt_h[0:1, 0, 0, 0:1])
            nc.vector.tensor_copy(scr2, pairt_h[0:1, 0, 1, 0:1])
            if hp + 1 < H // 2:
                fetch_pair(hp + 1)

            ov0 = psO.tile([65, R], F32, tag="ov")
            ov1 = psO.tile([65, R], F32, tag="ov")
            a_prev = None
            for kc in range(KC):
                st = psA.tile([128, 2, R], F32, tag="ps")
                nc.tensor.matmul(
                    st[:, 0, :], kT[0:64, hp, kc * 128:(kc + 1) * 128],
                    qT[0:64, hp, :], start=True, stop=True)
                nc.tensor.matmul(
                    st[:, 1, :], kT[64:128, hp, kc * 128:(kc + 1) * 128],
                    qT[64:128, hp, :], start=True, stop=True)
                if kc == 1 and pending is not None:
                    emit_epilogue(*pending)
                    pending = None
                e = sb.tile([128, 2, R], BF16, tag="e")
                nc.scalar.activation(e, st, Exp)
                a_t = sb.tile([128, 2, R], BF16, tag="at")
                nc.vector.tensor_mul(a_t, e, pairt_h[:, kc, :, :])
                if a_prev is not None:
                    pk = kc - 1
                    nc.tensor.matmul(ov0, vS[:, pk, h0, :], a_prev[:, 0, :],
                                     start=(pk == 0), stop=False)
                    nc.tensor.matmul(ov1, vS[:, pk, h0 + 1, :],
                                     a_prev[:, 1, :],
                                     start=(pk == 0), stop=False)
                a_prev = a_t
            nc.tensor.matmul(ov0, vS[:, KC - 1, h0, :], a_prev[:, 0, :],
                             start=False, stop=True)
            nc.tensor.matmul(ov1, vS[:, KC - 1, h0 + 1, :], a_prev[:, 1, :],
                             start=False, stop=True)

            dinvs = []
            for hi, ov in ((0, ov0), (1, ov1)):
                dinv = dp.tile([1, R], BF16, tag="dinv",
                               name=f"dinv_{hp}_{hi}")
                with nc.allow_low_precision(reason="1/d bf16 for bcast mm"):
                    nc.vector.reciprocal(dinv, ov[64:65, :])
                dinvs.append(dinv)
                if hp == H // 2 - 1:
                    # last pair: emit this head's epilogue immediately so the
                    # chain overlaps the other head's reciprocal
                    po = hi * 64
                    db = psD.tile([64, R], F32, tag="db",
                                  name=f"db_l_{hi}")
                    nc.tensor.matmul(db, ones_64, dinv, start=True, stop=True)
                    gd = dp.tile([64, R], F32, tag="gd", name=f"gd_l_{hi}")
                    nc.vector.tensor_mul(gd, gT[po:po + 64, hp, :], db)
                    nc.vector.tensor_mul(xgT[po:po + 64, hp, :],
                                         ov[0:64, :], gd)
            if hp < H // 2 - 1:
                pending = (hp, ov0, ov1, dinvs)

        # ---- output projection + bias ----
        nc.tensor.matmul(tps[0:1, 4:5], w_sb["wo"][0:1, 0, 0:1],
                         w_sb["wo"][0:1, 0, 0:1], start=True, stop=True)
        out_r = out.rearrange("(cc p) r -> cc p r", p=128)
        for cc in range(CC):
            ps = psA.tile([128, R], F32, tag="ps")
            for ci in range(CC):
                nc.tensor.matmul(
                    ps, w_sb["wo"][:, ci, cc * 128:(cc + 1) * 128],
                    xgT[:, ci, :], start=(ci == 0), stop=(ci == CC - 1))
            osb = outp.tile([128, R], BF16, tag="out")
            nc.scalar.activation(osb, ps, Ident, bias=bor_sb[:, cc:cc + 1])
            nc.sync.dma_start(out=out_r[cc], in_=osb)


def prep_in_maps(q_x, kv_x, bias_mask, bias_pair, Wq, Wk, Wv, Wg, bg, Wo, bo):
    f32 = np.float32
    shared = {
        "wq": np.ascontiguousarray((np.asarray(Wq, f32) * 0.125).astype(BF)),
        "wk": np.ascontiguousarray(np.asarray(Wk, f32).astype(BF)),
        "wv": np.ascontiguousarray(np.asarray(Wv, f32).astype(BF)),
        "wg": np.ascontiguousarray(np.asarray(Wg, f32).astype(BF)),
        "wo": np.ascontiguousarray(np.asarray(Wo, f32).astype(BF)),
        "bgr": np.ascontiguousarray(
            np.asarray(bg, f32).reshape(CC, 128).T),
        "bor": np.ascontiguousarray(
            np.asarray(bo, f32).reshape(CC, 128).T),
    }
    pair_exp_t = {}
    bp = np.asarray(bias_pair, f32)[0]  # [H, N, N] (h, q, k)
    for qh in (0, 1):
        sl = bp[:, qh * R:(qh + 1) * R, :]          # [H, R(q), N(k)]
        pair_exp_t[qh] = np.ascontiguousarray(
            np.exp(sl).transpose(0, 2, 1).astype(BF))  # [H, N(k), R(q)]

    in_maps = []
    for i in range(8):
        b, qh = i // 2, i % 2
        m = dict(shared)
        m["qxt"] = np.ascontiguousarray(
            np.asarray(q_x[b, qh * R:(qh + 1) * R, :], f32).T.astype(BF))
        m["kvt"] = np.ascontiguousarray(np.asarray(kv_x[b], f32).T.astype(BF))
        em = np.exp(np.asarray(bias_mask[b, 0, 0], f32))
        m["kvem"] = np.ascontiguousarray(
            (np.asarray(kv_x[b], f32) * em[:, None]).T.astype(BF))
        m["emb"] = np.ascontiguousarray(em.reshape(KC, 128).T.astype(BF))
        m["pairt"] = pair_exp_t[qh]
        in_maps.append(m)
    return in_maps


def assemble(results):
    out = np.empty((B, N, C), np.float32)
    for i, r in enumerate(results):
        b, qh = i // 2, i % 2
        out[b, qh * R:(qh + 1) * R, :] = np.asarray(r["out"], np.float32).T
    return out


def kernel(q_x, kv_x, bias_mask, bias_pair, Wq, Wk, Wv, Wg, bg, Wo, bo):
    nc = build()
    in_maps = prep_in_maps(q_x, kv_x, bias_mask, bias_pair,
                           Wq, Wk, Wv, Wg, bg, Wo, bo)
    res = run_bass_kernel_spmd(nc, in_maps, core_ids=list(range(8)))
    return assemble(res.results)


if __name__ == "__main__":
    nc = build()
    print("build OK")



# revision 75
# speedup vs baseline: 1.3690x; 1.3690x over previous
"""Trainium2 Bass kernel for AlphaFold-style gated attention.

Reference math (B=4, N=1024, C=512, H=8, CH=64):
    q = (q_x @ Wq) / 8 ; k = kv_x @ Wk ; v = kv_x @ Wv
    s = q k^T + bias_mask[b,k] + bias_pair[h,q,k]
    a = softmax_k(s) ; o = a @ v
    g = sigmoid(q_x @ Wg + bg)
    out = (o*g) @ Wo + bo

Sharding: 8 cores = (batch b in 0..3) x (q-half qh in 0..1). Zero collectives.

Device-side design:
  - fp8(e4m3) DoubleRow matmuls for the q/g/k/v projections (2 contraction
    chunks per instruction at 0.5 cyc/row). Weights are sent x16 so their
    values sit in e4m3's normal range; the 1/16 (and the reference's 1/8)
    are folded into the exp/sigmoid activation scale.
  - q and v projections use residual-compensated fp8: x ~ x_hi + x_lo and
    16W ~ w_hi + w_lo (both residuals stored unscaled in e4m3), with the
    cross terms x_lo@w_hi + x_hi@w_lo summed by one DoubleRow matmul pair.
  - bias_mask folded into the per-partition bias of the Exp activation
    (scores live [k-part, q-free]), so no kvem / em tensors at all.
  - bias_pair as host-precomputed exp(pair)^T (bf16), multiplied into
    exp(s) on DVE in 2x mode.
  - AV flipped: out[q-part, 65-free] = a[k,q]^T @ [v | 16], so the matmul
    streams only 65 rows, the softmax denominator is the 65th column
    (v is at scale 16, so the denominator column is the constant 16),
    and 1/d + gating become cheap per-partition DVE ops.
  - o*g transposed back per head-pair with PE transposes; the output
    projection accumulates the first two head-pair slices during the last
    attention block, leaving only 2 matmuls + a fused bias/partial combine
    per output chunk in the tail.
  - Deep software pipeline: AV matmuls run LAG=5 k-chunks behind scores
    (crossing head-pair boundaries) so the in-order PE never stalls on the
    ACT->DVE chain; projections/epilogues are hand-interleaved into the
    per-k-chunk schedule; a share of the pair-multiplies runs on GPSIMD
    (which must never touch PSUM - hardware restriction).
  - PSUM: scores 2x[128,2,512] (4 banks), AV accumulators 2 banks,
    projections-then-outproj 2 banks.
"""

import sys

import numpy as np

if "/opt/trn_rl_repo" not in sys.path:
    sys.path.insert(0, "/opt/trn_rl_repo")

import ml_dtypes

import concourse.bass as bass
import concourse.tile as tile
from concourse import bacc, mybir
from concourse.bass_utils import run_bass_kernel_spmd

B, N, C, H, CH = 4, 1024, 512, 8, 64
R = 512          # q rows per core
KC = N // 128    # 8 k chunks of 128
CC = C // 128    # 4 feature chunks of 128
F32 = mybir.dt.float32
BF16 = mybir.dt.bfloat16
FP8 = mybir.dt.float8e4
BF = ml_dtypes.bfloat16
F8 = ml_dtypes.float8_e4m3
DR = mybir.MatmulPerfMode.DoubleRow
EXP_SCALE = 0.125 / 256.0  # reference 1/sqrt(64) and the two x16 weight scales


def build(finalize=True):
    nc = bacc.Bacc("TRN2", target_bir_lowering=False, debug=False)

    qx8 = nc.dram_tensor("qx8", [C, 2, R], FP8, kind="ExternalInput").ap()
    kv8 = nc.dram_tensor("kv8", [C, 2, N], FP8, kind="ExternalInput").ap()
    wq8 = nc.dram_tensor("wq8", [C, 2, C], FP8, kind="ExternalInput").ap()
    wk8 = nc.dram_tensor("wk8", [C, C], FP8, kind="ExternalInput").ap()
    wg8 = nc.dram_tensor("wg8", [C, C], FP8, kind="ExternalInput").ap()
    wv8 = nc.dram_tensor("wv8", [C, 2, C], FP8, kind="ExternalInput").ap()
    wob = nc.dram_tensor("wob", [C, C], BF16, kind="ExternalInput").ap()
    maskb = nc.dram_tensor("maskb", [128, KC], F32, kind="ExternalInput").ap()
    bg16r = nc.dram_tensor("bg16r", [1, C], BF16, kind="ExternalInput").ap()
    bor = nc.dram_tensor("bor", [128, CC], F32, kind="ExternalInput").ap()
    ident = nc.dram_tensor("ident", [128, 128], BF16, kind="ExternalInput").ap()
    pairt = nc.dram_tensor("pairt", [H, N, R], BF16, kind="ExternalInput").ap()
    out = nc.dram_tensor("out", [C, R], BF16, kind="ExternalOutput").ap()

    with tile.TileContext(nc) as tc:
        _body(tc, qx8, kv8, wq8, wk8, wg8, wv8, wob, maskb, bg16r, bor,
              ident, pairt, out)
    if finalize:
        nc.finalize()
    return nc


def _body(tc, qx8, kv8, wq8, wk8, wg8, wv8, wob, maskb, bg16r, bor,
          ident, pairt, out):
    nc = tc.nc
    Exp = mybir.ActivationFunctionType.Exp
    Sigmoid = mybir.ActivationFunctionType.Sigmoid

    with (
        tc.tile_pool(name="keep", bufs=1) as keep,
        tc.tile_pool(name="sb", bufs=8) as sb,
        tc.tile_pool(name="pairp", bufs=4) as pairp,
        tc.tile_pool(name="dp", bufs=2) as dp,
        tc.tile_pool(name="outp", bufs=4) as outp,
        tc.tile_pool(name="psA", bufs=2, space="PSUM") as psA,
        tc.tile_pool(name="psO", bufs=2, space="PSUM") as psO,
        tc.tile_pool(name="psB", bufs=2, space="PSUM") as psB,
    ):
        # ---- persistent SBUF tiles ----
        qx8_sb = keep.tile([128, CC, 2, R], FP8, tag="qx8")
        kv8_sb = keep.tile([128, CC, 2, N], FP8, tag="kv8")
        wq8_sb = keep.tile([128, CC, 2, C], FP8, tag="wq8")
        wk8_sb = keep.tile([128, CC, C], FP8, tag="wk8")
        wg8_sb = keep.tile([128, CC, C], FP8, tag="wg8")
        wv8_sb = keep.tile([128, CC, 2, C], FP8, tag="wv8")
        wob_sb = keep.tile([128, CC, C], BF16, tag="wob")
        maskb_sb = keep.tile([128, KC], F32, tag="maskb")
        bg16r_sb = keep.tile([1, C], BF16, tag="bg16r")
        bor_sb = keep.tile([128, CC], F32, tag="bor")
        ident_sb = keep.tile([128, 128], BF16, tag="ident")
        ones1 = keep.tile([1, 128], BF16, tag="ones1")

        qT = keep.tile([128, CC, R], BF16, tag="qT")
        kT = keep.tile([128, CC, N], BF16, tag="kT")
        gQ = keep.tile([128, CC, C], BF16, tag="gQ")
        vS = keep.tile([128, KC, H, 65], BF16, tag="vS")
        og = keep.tile([128, CC, C], BF16, tag="og")
        ogT = keep.tile([128, CC, R], BF16, tag="ogT")
        qT0hi = keep.tile([128, R], BF16, tag="qT0hi")
        dinv = keep.tile([128, 2, CC], F32, tag="dinv")

        # ---- input DMAs, hand-ordered by first-consumer time ----
        # Lead (first scores need qT + kT block0): hi planes of qx8/wq8,
        # then wk8 + kv8-hi.  The lo (residual) planes follow, then the
        # v-projection weights race the sliced pair0 stream.  Pair tiles
        # are sliced (head, 2 k-chunks) so consumption at one exp/kc
        # (~1.04us) never waits on a monolithic 2MB fetch.
        qx8_r = qx8.rearrange("(cc p) t r -> t p cc r", p=128)
        wq8_r = wq8.rearrange("(cc p) t o -> t p cc o", p=128)
        kv8_r = kv8.rearrange("(cc p) t n -> t p cc n", p=128)
        wk8_r = wk8.rearrange("(cc p) o -> p cc o", p=128)
        nc.sync.dma_start(out=qx8_sb[:, :, 1, :], in_=qx8_r[1])  # x hi
        nc.sync.dma_start(out=wq8_sb[:, :, 0, :], in_=wq8_r[0])  # w hi
        nc.sync.dma_start(out=wk8_sb[:, :, 0:128], in_=wk8_r[:, :, 0:128])
        nc.sync.dma_start(out=kv8_sb[:, :, 1, 0:512], in_=kv8_r[1][:, :, 0:512])
        nc.sync.dma_start(out=maskb_sb, in_=maskb)
        nc.sync.dma_start(out=wk8_sb[:, :, 128:512], in_=wk8_r[:, :, 128:512])
        nc.sync.dma_start(out=kv8_sb[:, :, 1, 512:1024], in_=kv8_r[1][:, :, 512:1024])
        nc.sync.dma_start(out=bg16r_sb, in_=bg16r)
        nc.sync.dma_start(out=qx8_sb[:, :, 0, :], in_=qx8_r[0])  # x lo
        nc.sync.dma_start(out=wq8_sb[:, :, 1, :], in_=wq8_r[1])  # w lo

        nc.vector.memset(ones1, 1.0)
        nc.vector.memset(vS[:, :, :, 64:65], 16.0)

        pair_tiles = {}
        pr = pairt.rearrange("h (kc p) r -> h p kc r", p=128)

        def fetch_pair_slice(hp, q):
            # one (kc-pair) slice for both heads of head-pair hp
            if hp not in pair_tiles:
                pair_tiles[hp] = pairp.tile([128, KC, 2, R], BF16, tag="pair",
                                            name=f"pairt_{hp}")
            t = pair_tiles[hp]
            for hi in range(2):
                nc.sync.dma_start(
                    out=t[:, 2 * q:2 * q + 2, hi, :],
                    in_=pr[2 * hp + hi][:, 2 * q:2 * q + 2, :])

        nc.sync.dma_start(out=wv8_sb, in_=wv8.rearrange("(cc p) t o -> p cc t o", p=128))
        nc.sync.dma_start(out=kv8_sb[:, :, 0, :], in_=kv8_r[0])  # x lo
        fetch_pair_slice(0, 0)
        fetch_pair_slice(0, 1)
        fetch_pair_slice(0, 2)
        nc.sync.dma_start(out=wg8_sb, in_=wg8.rearrange("(cc p) o -> p cc o", p=128))
        fetch_pair_slice(0, 3)
        fetch_pair_slice(1, 0)
        fetch_pair_slice(1, 1)
        nc.sync.dma_start(out=ident_sb, in_=ident)
        fetch_pair_slice(1, 2)
        fetch_pair_slice(1, 3)
        for q in range(4):
            fetch_pair_slice(2, q)
        nc.sync.dma_start(out=wob_sb, in_=wob.rearrange("(cc p) o -> p cc o", p=128))
        nc.sync.dma_start(out=bor_sb, in_=bor)
        for q in range(4):
            fetch_pair_slice(3, q)

        # ---- projection emitters (called from the interleave schedule) ----
        def q_hi(blk, pq, close=False):
            sl = slice(128 * blk, 128 * blk + 128)
            for qh in range(2):
                qsl = slice(256 * qh, 256 * qh + 256)
                for t in range(2):
                    nc.tensor.matmul(
                        pq[:, qsl], wq8_sb[:, 2 * t:2 * t + 2, 0, sl],
                        qx8_sb[:, 2 * t:2 * t + 2, 1, qsl],
                        start=(qh == 0 and t == 0),
                        stop=(close and qh == 1 and t == 1), perf_mode=DR)

        def q_corr(blk, pq):
            sl = slice(128 * blk, 128 * blk + 128)
            for qh in range(2):
                qsl = slice(256 * qh, 256 * qh + 256)
                for cc in range(CC):
                    nc.tensor.matmul(
                        pq[:, qsl], wq8_sb[:, cc, 0:2, sl],
                        qx8_sb[:, cc, 0:2, qsl],
                        start=False, stop=(qh == 1 and cc == CC - 1),
                        perf_mode=DR)
            nc.vector.tensor_copy(qT[:, blk, :], pq)

        def q_block(blk):
            # qT[:, blk, :] = 16 * (q_x @ Wq)[:, blk-slice].T  (residual fp8)
            pq = psB.tile([128, R], F32, tag="pb", name=f"pq_{blk}")
            q_hi(blk, pq)
            q_corr(blk, pq)

        def k_block(blk, nh):
            # kT[:, blk, nh-half] = 16 * (kv_x @ Wk)[nh, blk-slice].T (fp8 hi)
            pk = psB.tile([128, 512], F32, tag="pb", name=f"pk_{blk}_{nh}")
            sl = slice(128 * blk, 128 * blk + 128)
            for quarter in range(2):
                ksl = slice(512 * nh + 256 * quarter,
                            512 * nh + 256 * quarter + 256)
                osl = slice(256 * quarter, 256 * quarter + 256)
                for t in range(2):
                    nc.tensor.matmul(
                        pk[:, osl], wk8_sb[:, 2 * t:2 * t + 2, sl],
                        kv8_sb[:, 2 * t:2 * t + 2, 1, ksl],
                        start=(t == 0 and quarter == 0),
                        stop=(t == 1 and quarter == 1), perf_mode=DR)
            nc.vector.tensor_copy(kT[:, blk, 512 * nh:512 * nh + 512], pk)

        # v chunks split into hi and corr+copy halves emitted one slot
        # apart: halves the per-slot PE burst and decouples the kv8-lo DMA
        # arrival from the scores stream
        v_psum = {}

        def v_hi(kc):
            pv = psB.tile([128, C], F32, tag="pb", name=f"pv_{kc}")
            v_psum[kc] = pv
            ksl = slice(128 * kc, 128 * kc + 128)
            for hh in range(2):
                vsl = slice(256 * hh, 256 * hh + 256)
                for t in range(2):
                    nc.tensor.matmul(
                        pv[:, vsl], kv8_sb[:, 2 * t:2 * t + 2, 1, ksl],
                        wv8_sb[:, 2 * t:2 * t + 2, 0, vsl],
                        start=(t == 0 and hh == 0), stop=False, perf_mode=DR)

        def v_corr(kc):
            pv = v_psum.pop(kc)
            ksl = slice(128 * kc, 128 * kc + 128)
            for hh in range(2):
                vsl = slice(256 * hh, 256 * hh + 256)
                for cc in range(CC):
                    nc.tensor.matmul(
                        pv[:, vsl], kv8_sb[:, cc, 0:2, ksl],
                        wv8_sb[:, cc, 0:2, vsl],
                        start=False,
                        stop=(hh == 1 and cc == CC - 1), perf_mode=DR)
            nc.vector.tensor_copy(
                vS[:, kc, :, 0:64],
                pv.rearrange("p (h c) -> p h c", h=H))

        def g_chunk(qc):
            # gQ[:, qc, :] = sigmoid(q_x @ Wg + bg)[qc q-chunk]  ([q, hc])
            pg = psB.tile([128, C], F32, tag="pb", name=f"pg_{qc}")
            qsl = slice(128 * qc, 128 * qc + 128)
            for hh in range(2):
                osl = slice(256 * hh, 256 * hh + 256)
                for t in range(2):
                    nc.tensor.matmul(
                        pg[:, osl], qx8_sb[:, 2 * t:2 * t + 2, 1, qsl],
                        wg8_sb[:, 2 * t:2 * t + 2, osl],
                        start=(t == 0 and hh == 0), stop=False, perf_mode=DR)
            nc.tensor.matmul(pg, ones1, bg16r_sb,
                             start=False, stop=True)
            nc.scalar.activation(gQ[:, qc, :], pg, Sigmoid,
                                 scale=1.0 / 16.0)

        # ---- out-projection: partials over ci 0..2 accumulate during hp3
        # (reusing psB), drain to SBUF f32; the tail adds only the ci=3
        # matmul + a fused (psum+bias)+partial combine per cc ----
        po_sb = keep.tile([128, CC, R], F32, tag="po_sb")

        pacc = {}
        po_ps = {}

        def o_part(cc, keep_open=False):
            # partial out-projection: ci 0..1 only (ogT 2/3 land later)
            pa = psB.tile([128, R], F32, tag="pb", name=f"pacc_{cc}")
            pacc[cc] = pa
            for ci in range(2):
                nc.tensor.matmul(
                    pa, wob_sb[:, ci, 128 * cc:128 * cc + 128],
                    ogT[:, ci, :], start=(ci == 0),
                    stop=(ci == 1 and not keep_open))

        def o_drain(cc):
            nc.vector.tensor_copy(po_sb[:, cc, :], pacc[cc])

        out_r = out.rearrange("(cc p) r -> cc p r", p=128)

        # ---- interleave schedule: extra PE work after scores of (hp, kc) ----
        extra = {(hp, kc): [] for hp in range(H // 2) for kc in range(KC)}
        extra[(0, 0)].append(lambda: q_block(0))
        extra[(0, 1)].append(lambda: k_block(0, 1))
        extra[(0, 1)].append(lambda: q_block(1))
        for i in range(6):
            extra[(0, 2 + i)].append(lambda kc=i: v_hi(kc))
            extra[(0, 3 + i) if i < 5 else (1, 0)].append(
                lambda kc=i: v_corr(kc))
        extra[(0, 4)].append(lambda: k_block(1, 0))
        extra[(0, 5)].append(lambda: g_chunk(0))
        extra[(0, 6)].append(lambda: g_chunk(1))
        extra[(0, 7)].append(lambda: g_chunk(2))
        extra[(1, 0)].append(lambda: v_hi(6))
        extra[(1, 1)].append(lambda: v_corr(6))
        extra[(1, 0)].append(lambda: g_chunk(3))
        extra[(1, 1)].append(lambda: v_hi(7))
        extra[(1, 2)].append(lambda: v_corr(7))
        extra[(1, 2)].append(lambda: k_block(1, 1))
        extra[(1, 3)].append(lambda: k_block(2, 0))
        extra[(1, 4)].append(lambda: q_block(2))
        extra[(2, 0)].append(lambda: k_block(2, 1))
        extra[(2, 2)].append(lambda: k_block(3, 0))
        extra[(2, 3)].append(lambda: q_block(3))
        extra[(2, 4)].append(lambda: k_block(3, 1))
        # out-projection partials: only after the last psB projection tile
        # so the pool rotation stays acyclic; ci=2 (ogT of hp2) lands after
        # the hp2 epilogue at (3, kc==3)
        extra[(3, 1)].append(lambda: o_part(0))
        extra[(3, 2)].append(lambda: o_part(1))
        extra[(3, 3)].append(lambda: o_drain(0))
        extra[(3, 4)].append(lambda: o_drain(1))
        extra[(3, 4)].append(lambda: o_part(2, keep_open=True))
        extra[(3, 5)].append(lambda: o_part(3, keep_open=True))

        # ---- PE warmup: ramp the p-state clock while lead DMAs land ----
        wu = psA.tile([128, 2, R], F32, tag="st", name="wu")
        for _ in range(10):
            nc.tensor.matmul(wu[0:1, 0, 0:128], ones1[:, 0:1], ones1,
                             start=True, stop=True)

        # ---- lead-in: a hi-only q block-0 (used by hp0 kc<3) and k00 so
        # the first scores only wait on the four hi-plane DMAs ----
        pq0 = psB.tile([128, R], F32, tag="pb", name="pq_0v1")
        q_hi(0, pq0, close=True)
        nc.vector.tensor_copy(qT0hi, pq0)
        # k block0/nh0 in two quarter-groups so the first scores only wait
        # on the first 256 columns of kT
        pk0 = psB.tile([128, 512], F32, tag="pb", name="pk_0_0")
        for quarter in range(2):
            osl = slice(256 * quarter, 256 * quarter + 256)
            for t in range(2):
                nc.tensor.matmul(
                    pk0[:, osl], wk8_sb[:, 2 * t:2 * t + 2, 0:128],
                    kv8_sb[:, 2 * t:2 * t + 2, 1, osl],
                    start=(t == 0), stop=(t == 1), perf_mode=DR)
            nc.vector.tensor_copy(kT[:, 0, osl], pk0[:, osl])

        # ---- attention: head pairs, software-pipelined ----
        pending = None

        def emit_epilogue(hp, ov_pair, skip_recip=False):
            h0 = 2 * hp
            hsl = slice(128 * hp, 128 * hp + 128)
            if not skip_recip:
                for head, ov in ((0, ov_pair[0]), (1, ov_pair[1])):
                    with nc.allow_low_precision(reason="softmax denom recip"):
                        nc.vector.reciprocal(dinv[:, head, :], ov[:, :, 64])
            gd = dp.tile([128, CC, 128], BF16, tag="gd", name=f"gd_{hp}")
            dbc = bass.AP(
                tensor=dinv.tensor, offset=dinv.offset,
                ap=[list(p) for p in dinv.ap[:1]] + [[1, CC], [CC, 2], [0, 64]])
            nc.vector.tensor_mul(gd, gQ[:, :, hsl], dbc)
            for head, ov in ((0, ov_pair[0]), (1, ov_pair[1])):
                nc.vector.tensor_mul(
                    og[:, :, 128 * hp + 64 * head:128 * hp + 64 * head + 64],
                    ov[:, :, 0:64], gd[:, :, 64 * head:64 * head + 64])

        def emit_transp(hp):
            # transpose og[q, hp-slice] -> ogT[hc, q] via PE
            hsl = slice(128 * hp, 128 * hp + 128)
            pt = psA.tile([128, CC, 128], BF16, tag="st", name=f"pt_{hp}")
            for qc in range(CC):
                nc.tensor.matmul(
                    pt[:, qc, :], og[:, qc, hsl], ident_sb,
                    is_transpose=True, start=True, stop=True)
            nc.vector.tensor_copy(ogT[:, hp, :], pt.rearrange("p a b -> p (a b)"))

        def emit_av(hp, ov_pair, pk, a_pk):
            for head in range(2):
                for qc in range(CC):
                    nc.tensor.matmul(
                        ov_pair[head][:, qc, 0:65],
                        a_pk[:, head, 128 * qc:128 * qc + 128],
                        vS[:, pk, 2 * hp + head, :],
                        start=(pk == 0 and qc == 0),
                        stop=(pk == KC - 1 and qc == CC - 1))

        # AV runs LAG k-chunks behind scores (possibly crossing into the
        # next head-pair) so the PE never stalls on the ACT->DVE chain
        # that produces a_t
        LAG = 5
        a_hist = []
        for hp in range(H // 2):
            pairt_h = pair_tiles[hp]

            ov_pair = (
                psO.tile([128, CC, 65], F32, tag="ov", name=f"ov0_{hp}"),
                psO.tile([128, CC, 65], F32, tag="ov", name=f"ov1_{hp}"),
            )
            for kc in range(KC):
                st = psA.tile([128, 2, R], F32, tag="st")
                if hp == 0 and kc < 3:
                    q0, q1v = qT0hi[0:64, :], qT0hi[64:128, :]
                else:
                    q0, q1v = qT[0:64, hp, :], qT[64:128, hp, :]
                nc.tensor.matmul(
                    st[:, 0, :], kT[0:64, hp, 128 * kc:128 * kc + 128],
                    q0, start=True, stop=True)
                nc.tensor.matmul(
                    st[:, 1, :], kT[64:128, hp, 128 * kc:128 * kc + 128],
                    q1v, start=True, stop=True)
                for fn in extra[(hp, kc)]:
                    fn()
                if kc == 5 and pending is not None:
                    emit_epilogue(*pending)
                if kc == 7 and pending is not None:
                    emit_transp(pending[0])
                    pending = None
                e = sb.tile([128, 2, R], BF16, tag="e")
                nc.scalar.activation(e, st, Exp, bias=maskb_sb[:, kc:kc + 1],
                                     scale=EXP_SCALE)
                a_t = sb.tile([128, 2, R], BF16, tag="at")
                mul_eng = nc.gpsimd if kc in (0, 3, 6) else nc.vector
                mul_eng.tensor_mul(a_t, e, pairt_h[:, kc, :, :])
                a_hist.append((hp, ov_pair, kc, a_t))
                if len(a_hist) > LAG:
                    emit_av(*a_hist.pop(0))
            if hp < H // 2 - 1:
                pending = (hp, ov_pair)
            else:
                for item in a_hist:
                    emit_av(*item)
                a_hist = []
                # ci=2 out-projection contributions only need ogT[:, 2, :]
                # (ready) -- run them on the PE while the DVE epilogue drains
                for cc in (2, 3):
                    nc.tensor.matmul(
                        pacc[cc], wob_sb[:, 2, 128 * cc:128 * cc + 128],
                        ogT[:, 2, :], start=False, stop=False)
                emit_epilogue(hp, ov_pair)
                emit_transp(hp)

        # ---- finish output projection: only ci=3 and the combines are
        # left; cc2/cc3 combine on ACT, cc0/cc1 fuse bias+partial on DVE ----
        Add = mybir.AluOpType.add
        Ident = mybir.ActivationFunctionType.Identity
        for cc in (2, 3):
            pa = pacc[cc]
            nc.tensor.matmul(pa, wob_sb[:, 3, 128 * cc:128 * cc + 128],
                             ogT[:, 3, :], start=False, stop=True)
            osb = outp.tile([128, R], BF16, tag="out", name=f"osb_{cc}")
            nc.scalar.activation(osb, pa, Ident, bias=bor_sb[:, cc:cc + 1])
            (nc.gpsimd if cc == 2 else nc.scalar).dma_start(
                out=out_r[cc], in_=osb)
        for cc in (0, 1):
            ps = psA.tile([128, R], F32, tag="st", name=f"po_{cc}")
            nc.tensor.matmul(ps, wob_sb[:, 2, 128 * cc:128 * cc + 128],
                             ogT[:, 2, :], start=True, stop=False)
            nc.tensor.matmul(ps, wob_sb[:, 3, 128 * cc:128 * cc + 128],
                             ogT[:, 3, :], start=False, stop=True)
            osb = outp.tile([128, R], BF16, tag="out", name=f"osb_{cc}")
            nc.vector.scalar_tensor_tensor(osb, ps, bor_sb[:, cc:cc + 1],
                                           po_sb[:, cc, :], Add, Add)
            nc.sync.dma_start(out=out_r[cc], in_=osb)


def prep_in_maps(q_x, kv_x, bias_mask, bias_pair, Wq, Wk, Wv, Wg, bg, Wo, bo):
    f32 = np.float32
    S = np.float32(16.0)

    def res_pair_w(W):
        # weights: (hi, lo) stacked on dim 1, hi = f8(16W), lo = f8(16W - hi)
        w16 = np.asarray(W, f32) * S
        hi = w16.astype(F8)
        lo = (w16 - hi.astype(f32)).astype(F8)
        return np.ascontiguousarray(np.stack([hi, lo], axis=1))

    def res_pair_x(x):
        # activations: (lo, hi) on dim 1, hi = f8(x), lo = f8(x - hi)
        x = np.asarray(x, f32)
        hi = x.astype(F8)
        lo = (x - hi.astype(f32)).astype(F8)
        return np.ascontiguousarray(np.stack([lo, hi], axis=1))

    shared = {
        "wq8": res_pair_w(Wq),
        "wk8": np.ascontiguousarray((np.asarray(Wk, f32) * S).astype(F8)),
        "wg8": np.ascontiguousarray((np.asarray(Wg, f32) * S).astype(F8)),
        "wv8": res_pair_w(Wv),
        "wob": np.ascontiguousarray(np.asarray(Wo, f32).astype(BF)),
        "bg16r": np.ascontiguousarray(
            (np.asarray(bg, f32) * S).astype(BF).reshape(1, C)),
        "bor": np.ascontiguousarray(np.asarray(bo, f32).reshape(CC, 128).T),
        "ident": np.ascontiguousarray(np.eye(128, dtype=BF)),
    }
    pair_exp_t = {}
    bp = np.asarray(bias_pair, f32)[0]  # [H, N, N] (h, q, k)
    for qh in (0, 1):
        sl = bp[:, qh * R:(qh + 1) * R, :]          # [H, R(q), N(k)]
        pair_exp_t[qh] = np.ascontiguousarray(
            np.exp(sl).transpose(0, 2, 1).astype(BF))  # [H, N(k), R(q)]

    in_maps = []
    for i in range(8):
        b, qh = i // 2, i % 2
        m = dict(shared)
        m["qx8"] = res_pair_x(np.asarray(q_x[b, qh * R:(qh + 1) * R, :], f32).T)
        m["kv8"] = res_pair_x(np.asarray(kv_x[b], f32).T)
        m["maskb"] = np.ascontiguousarray(
            np.asarray(bias_mask[b, 0, 0], f32).reshape(KC, 128).T)
        m["pairt"] = pair_exp_t[qh]
        in_maps.append(m)
    return in_maps


def assemble(results):
    out = np.empty((B, N, C), np.float32)
    for i, r in enumerate(results):
        b, qh = i // 2, i % 2
        out[b, qh * R:(qh + 1) * R, :] = np.asarray(r["out"], np.float32).T
    return out


def kernel(q_x, kv_x, bias_mask, bias_pair, Wq, Wk, Wv, Wg, bg, Wo, bo):
    nc = build()
    in_maps = prep_in_maps(q_x, kv_x, bias_mask, bias_pair,
                           Wq, Wk, Wv, Wg, bg, Wo, bo)
    res = run_bass_kernel_spmd(nc, in_maps, core_ids=list(range(8)))
    return assemble(res.results)


if __name__ == "__main__":
    nc = build()
    print("build OK")


# revision 76
# speedup vs baseline: 1.3695x; 1.0003x over previous
"""Trainium2 Bass kernel for AlphaFold-style gated attention.

Reference math (B=4, N=1024, C=512, H=8, CH=64):
    q = (q_x @ Wq) / 8 ; k = kv_x @ Wk ; v = kv_x @ Wv
    s = q k^T + bias_mask[b,k] + bias_pair[h,q,k]
    a = softmax_k(s) ; o = a @ v
    g = sigmoid(q_x @ Wg + bg)
    out = (o*g) @ Wo + bo

Sharding: 8 cores = (batch b in 0..3) x (q-half qh in 0..1). Zero collectives.

Device-side design:
  - fp8(e4m3) DoubleRow matmuls for the q/g/k/v projections (2 contraction
    chunks per instruction at 0.5 cyc/row). Weights are sent x16 so their
    values sit in e4m3's normal range; the 1/16 (and the reference's 1/8)
    are folded into the exp/sigmoid activation scale.
  - q and v projections use residual-compensated fp8: x ~ x_hi + x_lo and
    16W ~ w_hi + w_lo (both residuals stored unscaled in e4m3), with the
    cross terms x_lo@w_hi + x_hi@w_lo summed by one DoubleRow matmul pair.
  - bias_mask folded into the per-partition bias of the Exp activation
    (scores live [k-part, q-free]), so no kvem / em tensors at all.
  - bias_pair as host-precomputed exp(pair)^T (bf16), multiplied into
    exp(s) on DVE in 2x mode.
  - AV flipped: out[q-part, 65-free] = a[k,q]^T @ [v | 16], so the matmul
    streams only 65 rows, the softmax denominator is the 65th column
    (v is at scale 16, so the denominator column is the constant 16),
    and 1/d + gating become cheap per-partition DVE ops.
  - o*g transposed back per head-pair with PE transposes; the output
    projection accumulates the first two head-pair slices during the last
    attention block, leaving only 2 matmuls + a fused bias/partial combine
    per output chunk in the tail.
  - Deep software pipeline: AV matmuls run LAG=5 k-chunks behind scores
    (crossing head-pair boundaries) so the in-order PE never stalls on the
    ACT->DVE chain; projections/epilogues are hand-interleaved into the
    per-k-chunk schedule; a share of the pair-multiplies runs on GPSIMD
    (which must never touch PSUM - hardware restriction).
  - PSUM: scores 2x[128,2,512] (4 banks), AV accumulators 2 banks,
    projections-then-outproj 2 banks.
"""

import sys

import numpy as np

if "/opt/trn_rl_repo" not in sys.path:
    sys.path.insert(0, "/opt/trn_rl_repo")

import ml_dtypes

import concourse.bass as bass
import concourse.tile as tile
from concourse import bacc, mybir
from concourse.bass_utils import run_bass_kernel_spmd

B, N, C, H, CH = 4, 1024, 512, 8, 64
R = 512          # q rows per core
KC = N // 128    # 8 k chunks of 128
CC = C // 128    # 4 feature chunks of 128
F32 = mybir.dt.float32
BF16 = mybir.dt.bfloat16
FP8 = mybir.dt.float8e4
BF = ml_dtypes.bfloat16
F8 = ml_dtypes.float8_e4m3
DR = mybir.MatmulPerfMode.DoubleRow
EXP_SCALE = 0.125 / 256.0  # reference 1/sqrt(64) and the two x16 weight scales


def build(finalize=True):
    nc = bacc.Bacc("TRN2", target_bir_lowering=False, debug=False)

    qx8 = nc.dram_tensor("qx8", [C, 2, R], FP8, kind="ExternalInput").ap()
    kv8 = nc.dram_tensor("kv8", [C, 2, N], FP8, kind="ExternalInput").ap()
    wq8 = nc.dram_tensor("wq8", [C, 2, C], FP8, kind="ExternalInput").ap()
    wk8 = nc.dram_tensor("wk8", [C, C], FP8, kind="ExternalInput").ap()
    wg8 = nc.dram_tensor("wg8", [C, C], FP8, kind="ExternalInput").ap()
    wv8 = nc.dram_tensor("wv8", [C, 2, C], FP8, kind="ExternalInput").ap()
    wob = nc.dram_tensor("wob", [C, C], BF16, kind="ExternalInput").ap()
    maskb = nc.dram_tensor("maskb", [128, KC], F32, kind="ExternalInput").ap()
    bg16r = nc.dram_tensor("bg16r", [1, C], BF16, kind="ExternalInput").ap()
    bor = nc.dram_tensor("bor", [128, CC], F32, kind="ExternalInput").ap()
    ident = nc.dram_tensor("ident", [128, 128], BF16, kind="ExternalInput").ap()
    pairt = nc.dram_tensor("pairt", [H, N, R], BF16, kind="ExternalInput").ap()
    out = nc.dram_tensor("out", [C, R], BF16, kind="ExternalOutput").ap()

    with tile.TileContext(nc) as tc:
        _body(tc, qx8, kv8, wq8, wk8, wg8, wv8, wob, maskb, bg16r, bor,
              ident, pairt, out)
    if finalize:
        nc.finalize()
    return nc


def _body(tc, qx8, kv8, wq8, wk8, wg8, wv8, wob, maskb, bg16r, bor,
          ident, pairt, out):
    nc = tc.nc
    Exp = mybir.ActivationFunctionType.Exp
    Sigmoid = mybir.ActivationFunctionType.Sigmoid

    with (
        tc.tile_pool(name="keep", bufs=1) as keep,
        tc.tile_pool(name="sb", bufs=8) as sb,
        tc.tile_pool(name="pairp", bufs=4) as pairp,
        tc.tile_pool(name="dp", bufs=2) as dp,
        tc.tile_pool(name="outp", bufs=4) as outp,
        tc.tile_pool(name="psA", bufs=2, space="PSUM") as psA,
        tc.tile_pool(name="psO", bufs=2, space="PSUM") as psO,
        tc.tile_pool(name="psB", bufs=2, space="PSUM") as psB,
    ):
        # ---- persistent SBUF tiles ----
        qx8_sb = keep.tile([128, CC, 2, R], FP8, tag="qx8")
        kv8_sb = keep.tile([128, CC, 2, N], FP8, tag="kv8")
        wq8_sb = keep.tile([128, CC, 2, C], FP8, tag="wq8")
        wk8_sb = keep.tile([128, CC, C], FP8, tag="wk8")
        wg8_sb = keep.tile([128, CC, C], FP8, tag="wg8")
        wv8_sb = keep.tile([128, CC, 2, C], FP8, tag="wv8")
        wob_sb = keep.tile([128, CC, C], BF16, tag="wob")
        maskb_sb = keep.tile([128, KC], F32, tag="maskb")
        bg16r_sb = keep.tile([1, C], BF16, tag="bg16r")
        bor_sb = keep.tile([128, CC], F32, tag="bor")
        ident_sb = keep.tile([128, 128], BF16, tag="ident")
        ones1 = keep.tile([1, 128], BF16, tag="ones1")

        qT = keep.tile([128, CC, R], BF16, tag="qT")
        kT = keep.tile([128, CC, N], BF16, tag="kT")
        gQ = keep.tile([128, CC, C], BF16, tag="gQ")
        vS = keep.tile([128, KC, H, 65], BF16, tag="vS")
        og = keep.tile([128, CC, C], BF16, tag="og")
        ogT = keep.tile([128, CC, R], BF16, tag="ogT")
        qT0hi = keep.tile([128, R], BF16, tag="qT0hi")
        dinv = keep.tile([128, 2, CC], F32, tag="dinv")

        # ---- input DMAs, hand-ordered by first-consumer time ----
        # Lead (first scores need qT + kT block0): hi planes of qx8/wq8,
        # then wk8 + kv8-hi.  The lo (residual) planes follow, then the
        # v-projection weights race the sliced pair0 stream.  Pair tiles
        # are sliced (head, 2 k-chunks) so consumption at one exp/kc
        # (~1.04us) never waits on a monolithic 2MB fetch.
        qx8_r = qx8.rearrange("(cc p) t r -> t p cc r", p=128)
        wq8_r = wq8.rearrange("(cc p) t o -> t p cc o", p=128)
        kv8_r = kv8.rearrange("(cc p) t n -> t p cc n", p=128)
        wk8_r = wk8.rearrange("(cc p) o -> p cc o", p=128)
        nc.sync.dma_start(out=qx8_sb[:, :, 1, :], in_=qx8_r[1])  # x hi
        nc.sync.dma_start(out=wq8_sb[:, :, 0, :], in_=wq8_r[0])  # w hi
        nc.sync.dma_start(out=wk8_sb[:, :, 0:128], in_=wk8_r[:, :, 0:128])
        nc.sync.dma_start(out=kv8_sb[:, :, 1, 0:512], in_=kv8_r[1][:, :, 0:512])
        nc.sync.dma_start(out=maskb_sb, in_=maskb)
        nc.sync.dma_start(out=wk8_sb[:, :, 128:512], in_=wk8_r[:, :, 128:512])
        nc.sync.dma_start(out=kv8_sb[:, :, 1, 512:1024], in_=kv8_r[1][:, :, 512:1024])
        nc.sync.dma_start(out=bg16r_sb, in_=bg16r)
        nc.sync.dma_start(out=qx8_sb[:, :, 0, :], in_=qx8_r[0])  # x lo
        nc.sync.dma_start(out=wq8_sb[:, :, 1, :], in_=wq8_r[1])  # w lo

        nc.vector.memset(ones1, 1.0)
        nc.vector.memset(vS[:, :, :, 64:65], 16.0)

        pair_tiles = {}
        pr = pairt.rearrange("h (kc p) r -> h p kc r", p=128)

        def fetch_pair_slice(hp, q):
            # one (kc-pair) slice for both heads of head-pair hp
            if hp not in pair_tiles:
                pair_tiles[hp] = pairp.tile([128, KC, 2, R], BF16, tag="pair",
                                            name=f"pairt_{hp}")
            t = pair_tiles[hp]
            for hi in range(2):
                nc.sync.dma_start(
                    out=t[:, 2 * q:2 * q + 2, hi, :],
                    in_=pr[2 * hp + hi][:, 2 * q:2 * q + 2, :])

        nc.sync.dma_start(out=wv8_sb, in_=wv8.rearrange("(cc p) t o -> p cc t o", p=128))
        nc.sync.dma_start(out=kv8_sb[:, :, 0, :], in_=kv8_r[0])  # x lo
        fetch_pair_slice(0, 0)
        fetch_pair_slice(0, 1)
        fetch_pair_slice(0, 2)
        nc.sync.dma_start(out=wg8_sb, in_=wg8.rearrange("(cc p) o -> p cc o", p=128))
        fetch_pair_slice(0, 3)
        fetch_pair_slice(1, 0)
        fetch_pair_slice(1, 1)
        nc.sync.dma_start(out=ident_sb, in_=ident)
        fetch_pair_slice(1, 2)
        fetch_pair_slice(1, 3)
        for q in range(4):
            fetch_pair_slice(2, q)
        nc.sync.dma_start(out=wob_sb, in_=wob.rearrange("(cc p) o -> p cc o", p=128))
        nc.sync.dma_start(out=bor_sb, in_=bor)
        for q in range(4):
            fetch_pair_slice(3, q)

        # ---- projection emitters (called from the interleave schedule) ----
        def q_hi(blk, pq, close=False):
            sl = slice(128 * blk, 128 * blk + 128)
            for qh in range(2):
                qsl = slice(256 * qh, 256 * qh + 256)
                for t in range(2):
                    nc.tensor.matmul(
                        pq[:, qsl], wq8_sb[:, 2 * t:2 * t + 2, 0, sl],
                        qx8_sb[:, 2 * t:2 * t + 2, 1, qsl],
                        start=(qh == 0 and t == 0),
                        stop=(close and qh == 1 and t == 1), perf_mode=DR)

        def q_corr(blk, pq):
            sl = slice(128 * blk, 128 * blk + 128)
            for qh in range(2):
                qsl = slice(256 * qh, 256 * qh + 256)
                for cc in range(CC):
                    nc.tensor.matmul(
                        pq[:, qsl], wq8_sb[:, cc, 0:2, sl],
                        qx8_sb[:, cc, 0:2, qsl],
                        start=False, stop=(qh == 1 and cc == CC - 1),
                        perf_mode=DR)
            nc.vector.tensor_copy(qT[:, blk, :], pq)

        def q_block(blk):
            # qT[:, blk, :] = 16 * (q_x @ Wq)[:, blk-slice].T  (residual fp8)
            pq = psB.tile([128, R], F32, tag="pb", name=f"pq_{blk}")
            q_hi(blk, pq)
            q_corr(blk, pq)

        def k_block(blk, nh):
            # kT[:, blk, nh-half] = 16 * (kv_x @ Wk)[nh, blk-slice].T (fp8 hi)
            pk = psB.tile([128, 512], F32, tag="pb", name=f"pk_{blk}_{nh}")
            sl = slice(128 * blk, 128 * blk + 128)
            for quarter in range(2):
                ksl = slice(512 * nh + 256 * quarter,
                            512 * nh + 256 * quarter + 256)
                osl = slice(256 * quarter, 256 * quarter + 256)
                for t in range(2):
                    nc.tensor.matmul(
                        pk[:, osl], wk8_sb[:, 2 * t:2 * t + 2, sl],
                        kv8_sb[:, 2 * t:2 * t + 2, 1, ksl],
                        start=(t == 0 and quarter == 0),
                        stop=(t == 1 and quarter == 1), perf_mode=DR)
            nc.vector.tensor_copy(kT[:, blk, 512 * nh:512 * nh + 512], pk)

        # v chunks split into hi and corr+copy halves emitted one slot
        # apart: halves the per-slot PE burst and decouples the kv8-lo DMA
        # arrival from the scores stream
        v_psum = {}

        def v_hi(kc):
            pv = psB.tile([128, C], F32, tag="pb", name=f"pv_{kc}")
            v_psum[kc] = pv
            ksl = slice(128 * kc, 128 * kc + 128)
            for hh in range(2):
                vsl = slice(256 * hh, 256 * hh + 256)
                for t in range(2):
                    nc.tensor.matmul(
                        pv[:, vsl], kv8_sb[:, 2 * t:2 * t + 2, 1, ksl],
                        wv8_sb[:, 2 * t:2 * t + 2, 0, vsl],
                        start=(t == 0 and hh == 0), stop=False, perf_mode=DR)

        def v_corr(kc):
            pv = v_psum.pop(kc)
            ksl = slice(128 * kc, 128 * kc + 128)
            for hh in range(2):
                vsl = slice(256 * hh, 256 * hh + 256)
                for cc in range(CC):
                    nc.tensor.matmul(
                        pv[:, vsl], kv8_sb[:, cc, 0:2, ksl],
                        wv8_sb[:, cc, 0:2, vsl],
                        start=False,
                        stop=(hh == 1 and cc == CC - 1), perf_mode=DR)
            nc.vector.tensor_copy(
                vS[:, kc, :, 0:64],
                pv.rearrange("p (h c) -> p h c", h=H))

        def g_chunk(qc):
            # gQ[:, qc, :] = sigmoid(q_x @ Wg + bg)[qc q-chunk]  ([q, hc])
            pg = psB.tile([128, C], F32, tag="pb", name=f"pg_{qc}")
            qsl = slice(128 * qc, 128 * qc + 128)
            for hh in range(2):
                osl = slice(256 * hh, 256 * hh + 256)
                for t in range(2):
                    nc.tensor.matmul(
                        pg[:, osl], qx8_sb[:, 2 * t:2 * t + 2, 1, qsl],
                        wg8_sb[:, 2 * t:2 * t + 2, osl],
                        start=(t == 0 and hh == 0), stop=False, perf_mode=DR)
            nc.tensor.matmul(pg, ones1, bg16r_sb,
                             start=False, stop=True)
            nc.scalar.activation(gQ[:, qc, :], pg, Sigmoid,
                                 scale=1.0 / 16.0)

        # ---- out-projection: partials over ci 0..2 accumulate during hp3
        # (reusing psB), drain to SBUF f32; the tail adds only the ci=3
        # matmul + a fused (psum+bias)+partial combine per cc ----
        po_sb = keep.tile([128, CC, R], F32, tag="po_sb")

        pacc = {}
        po_ps = {}

        def o_part(cc, keep_open=False):
            # partial out-projection: ci 0..1 only (ogT 2/3 land later)
            pa = psB.tile([128, R], F32, tag="pb", name=f"pacc_{cc}")
            pacc[cc] = pa
            for ci in range(2):
                nc.tensor.matmul(
                    pa, wob_sb[:, ci, 128 * cc:128 * cc + 128],
                    ogT[:, ci, :], start=(ci == 0),
                    stop=(ci == 1 and not keep_open))

        def o_drain(cc):
            nc.vector.tensor_copy(po_sb[:, cc, :], pacc[cc])

        out_r = out.rearrange("(cc p) r -> cc p r", p=128)

        # ---- interleave schedule: extra PE work after scores of (hp, kc) ----
        extra = {(hp, kc): [] for hp in range(H // 2) for kc in range(KC)}
        extra[(0, 0)].append(lambda: q_block(0))
        extra[(0, 1)].append(lambda: k_block(0, 1))
        extra[(0, 1)].append(lambda: q_block(1))
        for i in range(6):
            extra[(0, 2 + i)].append(lambda kc=i: v_hi(kc))
            extra[(0, 3 + i) if i < 5 else (1, 0)].append(
                lambda kc=i: v_corr(kc))
        extra[(0, 4)].append(lambda: k_block(1, 0))
        extra[(0, 5)].append(lambda: g_chunk(0))
        extra[(0, 6)].append(lambda: g_chunk(1))
        extra[(0, 7)].append(lambda: g_chunk(2))
        extra[(1, 0)].append(lambda: v_hi(6))
        extra[(1, 1)].append(lambda: v_corr(6))
        extra[(1, 0)].append(lambda: g_chunk(3))
        extra[(1, 1)].append(lambda: v_hi(7))
        extra[(1, 2)].append(lambda: v_corr(7))
        extra[(1, 2)].append(lambda: k_block(1, 1))
        extra[(1, 3)].append(lambda: k_block(2, 0))
        extra[(1, 4)].append(lambda: q_block(2))
        extra[(2, 0)].append(lambda: k_block(2, 1))
        extra[(2, 2)].append(lambda: k_block(3, 0))
        extra[(2, 3)].append(lambda: q_block(3))
        extra[(2, 4)].append(lambda: k_block(3, 1))
        # out-projection partials: only after the last psB projection tile
        # so the pool rotation stays acyclic; ci=2 (ogT of hp2) lands after
        # the hp2 epilogue at (3, kc==3)
        extra[(3, 1)].append(lambda: o_part(0))
        extra[(3, 2)].append(lambda: o_part(1))
        extra[(3, 3)].append(lambda: o_drain(0))
        extra[(3, 4)].append(lambda: o_drain(1))
        extra[(3, 4)].append(lambda: o_part(2, keep_open=True))
        extra[(3, 5)].append(lambda: o_part(3, keep_open=True))

        # ---- PE warmup: ramp the p-state clock while lead DMAs land ----
        wu = psA.tile([128, 2, R], F32, tag="st", name="wu")
        for _ in range(10):
            nc.tensor.matmul(wu[0:1, 0, 0:128], ones1[:, 0:1], ones1,
                             start=True, stop=True)

        # ---- lead-in: a hi-only q block-0 (used by hp0 kc<3) and k00 so
        # the first scores only wait on the four hi-plane DMAs ----
        pq0 = psB.tile([128, R], F32, tag="pb", name="pq_0v1")
        q_hi(0, pq0, close=True)
        nc.vector.tensor_copy(qT0hi, pq0)
        # k block0/nh0 in two quarter-groups so the first scores only wait
        # on the first 256 columns of kT
        pk0 = psB.tile([128, 512], F32, tag="pb", name="pk_0_0")
        for quarter in range(2):
            osl = slice(256 * quarter, 256 * quarter + 256)
            for t in range(2):
                nc.tensor.matmul(
                    pk0[:, osl], wk8_sb[:, 2 * t:2 * t + 2, 0:128],
                    kv8_sb[:, 2 * t:2 * t + 2, 1, osl],
                    start=(t == 0), stop=(t == 1), perf_mode=DR)
            nc.vector.tensor_copy(kT[:, 0, osl], pk0[:, osl])

        # ---- attention: head pairs, software-pipelined ----
        pending = None

        def emit_epilogue(hp, ov_pair, skip_recip=False):
            h0 = 2 * hp
            hsl = slice(128 * hp, 128 * hp + 128)
            if not skip_recip:
                for head, ov in ((0, ov_pair[0]), (1, ov_pair[1])):
                    with nc.allow_low_precision(reason="softmax denom recip"):
                        nc.vector.reciprocal(dinv[:, head, :], ov[:, :, 64])
            gd = dp.tile([128, CC, 128], BF16, tag="gd", name=f"gd_{hp}")
            dbc = bass.AP(
                tensor=dinv.tensor, offset=dinv.offset,
                ap=[list(p) for p in dinv.ap[:1]] + [[1, CC], [CC, 2], [0, 64]])
            nc.vector.tensor_mul(gd, gQ[:, :, hsl], dbc)
            for head, ov in ((0, ov_pair[0]), (1, ov_pair[1])):
                nc.vector.tensor_mul(
                    og[:, :, 128 * hp + 64 * head:128 * hp + 64 * head + 64],
                    ov[:, :, 0:64], gd[:, :, 64 * head:64 * head + 64])

        def emit_transp(hp):
            # transpose og[q, hp-slice] -> ogT[hc, q] via PE
            hsl = slice(128 * hp, 128 * hp + 128)
            pt = psA.tile([128, CC, 128], BF16, tag="st", name=f"pt_{hp}")
            for qc in range(CC):
                nc.tensor.matmul(
                    pt[:, qc, :], og[:, qc, hsl], ident_sb,
                    is_transpose=True, start=True, stop=True)
            nc.vector.tensor_copy(ogT[:, hp, :], pt.rearrange("p a b -> p (a b)"))

        def emit_av(hp, ov_pair, pk, a_pk):
            for head in range(2):
                for qc in range(CC):
                    nc.tensor.matmul(
                        ov_pair[head][:, qc, 0:65],
                        a_pk[:, head, 128 * qc:128 * qc + 128],
                        vS[:, pk, 2 * hp + head, :],
                        start=(pk == 0 and qc == 0),
                        stop=(pk == KC - 1 and qc == CC - 1))

        # AV runs LAG k-chunks behind scores (possibly crossing into the
        # next head-pair) so the PE never stalls on the ACT->DVE chain
        # that produces a_t
        LAG = 5
        a_hist = []
        for hp in range(H // 2):
            pairt_h = pair_tiles[hp]

            ov_pair = (
                psO.tile([128, CC, 65], F32, tag="ov", name=f"ov0_{hp}"),
                psO.tile([128, CC, 65], F32, tag="ov", name=f"ov1_{hp}"),
            )
            for kc in range(KC):
                st = psA.tile([128, 2, R], F32, tag="st")
                if hp == 0 and kc < 3:
                    q0, q1v = qT0hi[0:64, :], qT0hi[64:128, :]
                else:
                    q0, q1v = qT[0:64, hp, :], qT[64:128, hp, :]
                nc.tensor.matmul(
                    st[:, 0, :], kT[0:64, hp, 128 * kc:128 * kc + 128],
                    q0, start=True, stop=True)
                nc.tensor.matmul(
                    st[:, 1, :], kT[64:128, hp, 128 * kc:128 * kc + 128],
                    q1v, start=True, stop=True)
                for fn in extra[(hp, kc)]:
                    fn()
                if kc == 5 and pending is not None:
                    emit_epilogue(*pending)
                if kc == 7 and pending is not None:
                    emit_transp(pending[0])
                    pending = None
                e = sb.tile([128, 2, R], BF16, tag="e")
                a_t = sb.tile([128, 2, R], BF16, tag="at")
                if hp == H // 2 - 1 and kc == KC - 1:
                    # last chunk: per-head exp/mult so the tail epilogue can
                    # start on head0 while head1 is still in flight
                    for hd in range(2):
                        nc.scalar.activation(e[:, hd, :], st[:, hd, :], Exp,
                                             bias=maskb_sb[:, kc:kc + 1],
                                             scale=EXP_SCALE)
                        nc.vector.tensor_mul(a_t[:, hd, :], e[:, hd, :],
                                             pairt_h[:, kc, hd, :])
                else:
                    nc.scalar.activation(e, st, Exp,
                                         bias=maskb_sb[:, kc:kc + 1],
                                         scale=EXP_SCALE)
                    mul_eng = nc.gpsimd if kc in (0, 3, 6) else nc.vector
                    mul_eng.tensor_mul(a_t, e, pairt_h[:, kc, :, :])
                a_hist.append((hp, ov_pair, kc, a_t))
                if len(a_hist) > LAG:
                    emit_av(*a_hist.pop(0))
            if hp < H // 2 - 1:
                pending = (hp, ov_pair)
            else:
                for item in a_hist:
                    emit_av(*item)
                a_hist = []
                # ci=2 out-projection contributions only need ogT[:, 2, :]
                # (ready) -- run them on the PE while the DVE epilogue drains
                for cc in (2, 3):
                    nc.tensor.matmul(
                        pacc[cc], wob_sb[:, 2, 128 * cc:128 * cc + 128],
                        ogT[:, 2, :], start=False, stop=False)
                emit_epilogue(hp, ov_pair)
                emit_transp(hp)

        # ---- finish output projection: only ci=3 and the combines are
        # left; cc2/cc3 combine on ACT, cc0/cc1 fuse bias+partial on DVE ----
        Add = mybir.AluOpType.add
        Ident = mybir.ActivationFunctionType.Identity
        for cc in (2, 3):
            pa = pacc[cc]
            nc.tensor.matmul(pa, wob_sb[:, 3, 128 * cc:128 * cc + 128],
                             ogT[:, 3, :], start=False, stop=True)
            osb = outp.tile([128, R], BF16, tag="out", name=f"osb_{cc}")
            nc.scalar.activation(osb, pa, Ident, bias=bor_sb[:, cc:cc + 1])
            (nc.gpsimd if cc == 2 else nc.scalar).dma_start(
                out=out_r[cc], in_=osb)
        for cc in (0, 1):
            ps = psA.tile([128, R], F32, tag="st", name=f"po_{cc}")
            nc.tensor.matmul(ps, wob_sb[:, 2, 128 * cc:128 * cc + 128],
                             ogT[:, 2, :], start=True, stop=False)
            nc.tensor.matmul(ps, wob_sb[:, 3, 128 * cc:128 * cc + 128],
                             ogT[:, 3, :], start=False, stop=True)
            osb = outp.tile([128, R], BF16, tag="out", name=f"osb_{cc}")
            nc.vector.scalar_tensor_tensor(osb, ps, bor_sb[:, cc:cc + 1],
                                           po_sb[:, cc, :], Add, Add)
            nc.sync.dma_start(out=out_r[cc], in_=osb)


def prep_in_maps(q_x, kv_x, bias_mask, bias_pair, Wq, Wk, Wv, Wg, bg, Wo, bo):
    f32 = np.float32
    S = np.float32(16.0)

    def res_pair_w(W):
        # weights: (hi, lo) stacked on dim 1, hi = f8(16W), lo = f8(16W - hi)
        w16 = np.asarray(W, f32) * S
        hi = w16.astype(F8)
        lo = (w16 - hi.astype(f32)).astype(F8)
        return np.ascontiguousarray(np.stack([hi, lo], axis=1))

    def res_pair_x(x):
        # activations: (lo, hi) on dim 1, hi = f8(x), lo = f8(x - hi)
        x = np.asarray(x, f32)
        hi = x.astype(F8)
        lo = (x - hi.astype(f32)).astype(F8)
        return np.ascontiguousarray(np.stack([lo, hi], axis=1))

    shared = {
        "wq8": res_pair_w(Wq),
        "wk8": np.ascontiguousarray((np.asarray(Wk, f32) * S).astype(F8)),
        "wg8": np.ascontiguousarray((np.asarray(Wg, f32) * S).astype(F8)),
        "wv8": res_pair_w(Wv),
        "wob": np.ascontiguousarray(np.asarray(Wo, f32).astype(BF)),
        "bg16r": np.ascontiguousarray(
            (np.asarray(bg, f32) * S).astype(BF).reshape(1, C)),
        "bor": np.ascontiguousarray(np.asarray(bo, f32).reshape(CC, 128).T),
        "ident": np.ascontiguousarray(np.eye(128, dtype=BF)),
    }
    pair_exp_t = {}
    bp = np.asarray(bias_pair, f32)[0]  # [H, N, N] (h, q, k)
    for qh in (0, 1):
        sl = bp[:, qh * R:(qh + 1) * R, :]          # [H, R(q), N(k)]
        pair_exp_t[qh] = np.ascontiguousarray(
            np.exp(sl).transpose(0, 2, 1).astype(BF))  # [H, N(k), R(q)]

    in_maps = []
    for i in range(8):
        b, qh = i // 2, i % 2
        m = dict(shared)
        m["qx8"] = res_pair_x(np.asarray(q_x[b, qh * R:(qh + 1) * R, :], f32).T)
        m["kv8"] = res_pair_x(np.asarray(kv_x[b], f32).T)
        m["maskb"] = np.ascontiguousarray(
            np.asarray(bias_mask[b, 0, 0], f32).reshape(KC, 128).T)
        m["pairt"] = pair_exp_t[qh]
        in_maps.append(m)
    return in_maps


def assemble(results):
    out = np.empty((B, N, C), np.float32)
    for i, r in enumerate(results):
        b, qh = i // 2, i % 2
        out[b, qh * R:(qh + 1) * R, :] = np.asarray(r["out"], np.float32).T
    return out


def kernel(q_x, kv_x, bias_mask, bias_pair, Wq, Wk, Wv, Wg, bg, Wo, bo):
    nc = build()
    in_maps = prep_in_maps(q_x, kv_x, bias_mask, bias_pair,
                           Wq, Wk, Wv, Wg, bg, Wo, bo)
    res = run_bass_kernel_spmd(nc, in_maps, core_ids=list(range(8)))
    return assemble(res.results)


if __name__ == "__main__":
    nc = build()
    print("build OK")
